# revision 1
# baseline (speedup 1.0000x reference)
"""Trainium2 Bass kernel: batched single-channel 3x3 valid conv, 16 output channels.

reference: x [32, 512, 512] f32, kernels [16, 3, 3] f32
           -> out [32, 16, 510, 510] f32  (cross-correlation, VALID, stride 1)

Strategy (memory-regime problem: output is 532 MB, input 33 MB):
  - Data-parallel: 4 images per core across 8 cores; kernels replicated.
  - 30-row output blocks (510 = 17 x 30): per block one PE matmul per
    channel-group with contraction K = 3 column-shifts x 32 input rows = 96
    against a host-precomputed banded lhsT [96, 120] (M = 4 channels x 30
    rows). rhs [96, 510] is loaded by ONE input DMA whose source AP reads
    the 3 overlapping column windows (dims dx:3 step 1, y':32 step W, x:510).
  - PSUM [120, 510] tiles are copied (ScalarE/VectorE alternating) into
    per-channel-group staging tiles [120, 9*510]; each half-image flushes
    with per-channel ~0.6 MB dma_starts (128 output calls/core total) —
    large calls amortize the ~1.5 us per-dma_start overhead that dominates
    at small call sizes.
"""

import numpy as np

import concourse.bass as bass
import concourse.mybir as mybir
import concourse.tile as tile
from concourse import bacc
from concourse.bass_utils import run_bass_kernel_spmd

N_CORES = 8
B, H, W = 32, 512, 512
KN, KS = 16, 3
OH, OW = H - KS + 1, W - KS + 1  # 510, 510
B_LOC = B // N_CORES  # 4

ROWS = 30                # output rows per block (510 = 17 * 30)
IN_ROWS = ROWS + KS - 1  # 32 input rows per block
KDIM = KS * IN_ROWS      # 96 contraction
NBLK = OH // ROWS        # 17
KG = 4                   # channels per matmul group
N_GROUPS = KN // KG      # 4
M = KG * ROWS            # 120 psum partitions
HALVES = [(0, 9), (9, 8)]  # (first block, n blocks) per output flush

F32 = mybir.dt.float32


def _build_nc(use_f32r=False, in_ring="sync"):
    in_dt = mybir.dt.float32r if use_f32r else F32
    nc = bacc.Bacc("TRN2", target_bir_lowering=False, debug=False)
    x_t = nc.dram_tensor("x", [B_LOC, H, W], in_dt, kind="ExternalInput")
    w_t = nc.dram_tensor("w", [KDIM, N_GROUPS * M], in_dt, kind="ExternalInput")
    out_t = nc.dram_tensor("out", [B_LOC, KN, OH, OW], F32, kind="ExternalOutput")

    with tile.TileContext(nc) as tc:
        with (
            tc.tile_pool(name="wpool", bufs=1) as wpool,
            tc.tile_pool(name="inpool", bufs=4) as inpool,
            tc.tile_pool(name="psum", bufs=8, space="PSUM") as psum_pool,
            tc.tile_pool(name="stage", bufs=2) as stage_pool,
        ):
            wt = wpool.tile([KDIM, N_GROUPS * M], in_dt)
            nc.sync.dma_start(out=wt[:, :], in_=w_t[:, :])
            cp = 0
            for b in range(B_LOC):
                for h0, hn in HALVES:
                    bigs = [
                        stage_pool.tile(
                            [M, 9 * OW], F32, name=f"big{g}", tag=f"big{g}"
                        )
                        for g in range(N_GROUPS)
                    ]
                    for j in range(hn):
                        r = (h0 + j) * ROWS
                        base = inpool.tile([KDIM, OW], in_dt)
                        src = x_t.ap()[b]  # [H, W]
                        getattr(nc, in_ring).dma_start(
                            out=base[:, :],
                            in_=bass.AP(
                                src.tensor,
                                src.offset + r * W,
                                [[1, KS], [W, IN_ROWS], [1, OW]],
                            ),
                        )
                        for g in range(N_GROUPS):
                            ps = psum_pool.tile([M, OW], F32)
                            nc.tensor.matmul(
                                ps[:, :],
                                lhsT=wt[:, g * M : (g + 1) * M],
                                rhs=base[:, :],
                                start=True,
                                stop=True,
                            )
                            dst = bigs[g][:, j * OW : (j + 1) * OW]
                            if cp % 2 == 0:
                                nc.scalar.copy(out=dst, in_=ps[:, :])
                            else:
                                nc.vector.tensor_copy(out=dst, in_=ps[:, :])
                            cp += 1
                    for g in range(N_GROUPS):
                        for k in range(KG):
                            # DRAM dims (y:30, blk:hn, x:510) match SBUF
                            # (p=y, f=(blk, x))
                            view = out_t[
                                b,
                                g * KG + k,
                                h0 * ROWS : (h0 + hn) * ROWS,
                                :,
                            ].rearrange("(blk y) x -> y blk x", y=ROWS)
                            nc.sync.dma_start(
                                out=view,
                                in_=bigs[g][k * ROWS : (k + 1) * ROWS, 0 : hn * OW],
                            )
    nc.finalize()
    return nc


def _pack_weights(kernels: np.ndarray) -> np.ndarray:
    """lhsT pack: w[dx*IN_ROWS + y + dy, g*M + k*ROWS + y] = kernels[g*KG+k, dy, dx].

    psum[k*ROWS + y, n] = sum_{dx, y'} lhsT[dx*IN_ROWS + y', k*ROWS + y]
                                       * x[r + y', n + dx]
                        = sum_{dy, dx} kernels[g*KG+k, dy, dx] * x[r + y + dy, n + dx]
    """
    w = np.zeros((KDIM, N_GROUPS * M), np.float32)
    y = np.arange(ROWS)
    for g in range(N_GROUPS):
        for dx in range(KS):
            for k in range(KG):
                for dy in range(KS):
                    w[dx * IN_ROWS + y + dy, g * M + k * ROWS + y] = kernels[
                        g * KG + k, dy, dx
                    ]
    return w


def run(x, kernels, trace=False, use_f32r=False, **spmd_kwargs):
    x = np.ascontiguousarray(np.asarray(x, dtype=np.float32))
    kernels = np.asarray(kernels, dtype=np.float32)
    assert x.shape == (B, H, W) and kernels.shape == (KN, KS, KS)
    nc = _build_nc(use_f32r=use_f32r, **{k: v for k, v in spmd_kwargs.items() if k == "in_ring"})
    spmd_kwargs.pop("in_ring", None)
    wp = _pack_weights(kernels)
    in_maps = [
        {"x": x[c * B_LOC : (c + 1) * B_LOC], "w": wp} for c in range(N_CORES)
    ]
    res = run_bass_kernel_spmd(
        nc, in_maps, core_ids=list(range(N_CORES)), trace=trace, **spmd_kwargs
    )
    out = np.concatenate([res.results[c]["out"] for c in range(N_CORES)], axis=0)
    return out, res


def kernel(x, kernels):
    out, _ = run(x, kernels, trace=False)
    return out

